# revision 13
# baseline (speedup 1.0000x reference)
"""Cross-attention Trainium2 kernel (8 NeuronCores, SPMD).

Sharding: core -> (batch b = core//2, query-half h = core%2).
Each core computes out[b, :, h*2048:(h+1)*2048] independently:
  Qf = Wq @ X[b][:, half] + bq          [64, 2048]   (f32r)
  Kf = Wk @ Y[b] + bk                   [64, 4096]   (f32r)
  V0t = (Wv @ Y[b])^T                   [4096, 512]  (bf16)
  St[m, n] = sum_c Kf[c, m] Qf[c, n]    (f32r, scores transposed: no
                                         transposes needed anywhere)
  es = exp(St - 40)                     (bf16; the shift cancels in the
                                         normalization)
  AV[c, n] = sum_m V0t[m, c] es[m, n]   (bf16 matmul, fp32 PSUM)
  colsum[n] = sum_m es[m, n]            (DVE accumulate + GpSimd
                                         partition_all_reduce)
  out = AV * (gamma/colsum) + (X + gamma*bv)   (bv folds out of the
                                         attention since sum_m P = 1)
"""

import sys

if "/opt/trn_rl_repo" not in sys.path:
    sys.path.insert(0, "/opt/trn_rl_repo")

import ml_dtypes
import numpy as np

import concourse.bass as bass
import concourse.mybir as mybir
from concourse import bacc, bass_isa
from concourse.bass import ts
from concourse.bass_utils import run_bass_kernel_spmd
from concourse.tile import TileContext

F32 = mybir.dt.float32
F32R = mybir.dt.float32r
BF16 = mybir.dt.bfloat16
AF = mybir.ActivationFunctionType

B, C, H, W = 4, 512, 64, 64
CI = 64            # inner q/k dim
N = H * W          # 4096 key positions
NQ = N // 2        # 2048 query positions per core
NB = 512           # n-block (free dim per matmul)
MT = 128           # m-tile (key positions per scores tile)
NCHUNK = C // 128  # 4 channel chunks
NMT = N // MT      # 32 m-tiles
NQB = NQ // NB     # 4 query blocks per core
NG = 1024          # dma/rounding column group
EXP_SHIFT = -40.0


def _build():
    nc = bacc.Bacc(
        "TRN2", target_bir_lowering=False, debug=False, num_devices=8
    )
    xh = nc.declare_dram_parameter("xh", [C, NQ], F32, isOutput=False)
    yb = nc.declare_dram_parameter("yb", [C, N], F32, isOutput=False)
    wqt = nc.declare_dram_parameter("wqt", [C, CI], F32, isOutput=False)
    wkt = nc.declare_dram_parameter("wkt", [C, CI], F32, isOutput=False)
    wvt = nc.declare_dram_parameter("wvt", [C, C], BF16, isOutput=False)
    bq = nc.declare_dram_parameter("bq", [CI, 1], F32, isOutput=False)
    bk = nc.declare_dram_parameter("bk", [CI, 1], F32, isOutput=False)
    bv = nc.declare_dram_parameter("bv", [C, 1], F32, isOutput=False)
    gamma = nc.declare_dram_parameter("gamma", [1, 1], F32, isOutput=False)
    out = nc.declare_dram_parameter("out", [C, NQ], F32, isOutput=True)

    xh_r = xh.rearrange("(a p) n -> p a n", p=128)
    yb_r = yb.rearrange("(a p) n -> p a n", p=128)
    wqt_r = wqt.rearrange("(a p) m -> p a m", p=128)
    wkt_r = wkt.rearrange("(a p) m -> p a m", p=128)
    wvt_r = wvt.rearrange("(a p) m -> p a m", p=128)
    bv_r = bv.rearrange("(a p) o -> p a o", p=128)
    out_r = out.rearrange("(a p) n -> p a n", p=128)

    with TileContext(nc) as tc:
        with (
            tc.tile_pool(name="weights", bufs=1) as wp,
            tc.tile_pool(name="bigin", bufs=1) as bp,
            tc.tile_pool(name="psA", bufs=1, space="PSUM") as psA,
            tc.tile_pool(name="psB", bufs=4, space="PSUM") as psB,
        ):
            # ---- constants / weights ----
            wqt_s = wp.tile([128, NCHUNK, CI], F32)
            wkt_s = wp.tile([128, NCHUNK, CI], F32)
            wqt_rr = wp.tile([128, NCHUNK, CI], F32R)
            wkt_rr = wp.tile([128, NCHUNK, CI], F32R)
            wvt_s = wp.tile([128, NCHUNK, C], BF16)
            bq_s = wp.tile([CI, 1], F32)
            bk_s = wp.tile([CI, 1], F32)
            bv_s = wp.tile([128, NCHUNK, 1], F32)
            gb_s = wp.tile([128, 1], F32)
            gbv_s = wp.tile([128, NCHUNK, 1], F32)
            ebias_s = wp.tile([128, 1], F32)

            nc.sync.dma_start(out=wqt_s, in_=wqt_r)
            nc.sync.dma_start(out=wkt_s, in_=wkt_r)
            nc.sync.dma_start(out=wvt_s, in_=wvt_r)
            nc.sync.dma_start(out=bq_s, in_=bq[:, :])
            nc.sync.dma_start(out=bk_s, in_=bk[:, :])
            nc.sync.dma_start(out=bv_s, in_=bv_r)
            g_ap = gamma[:, :]
            gb_bcast = bass.AP(
                tensor=g_ap.tensor,
                offset=g_ap.offset,
                ap=[[0, 128]] + list(g_ap.ap)[1:],
            )
            nc.sync.dma_start(out=gb_s, in_=gb_bcast)
            nc.vector.tensor_copy(wqt_rr, wqt_s)
            nc.vector.tensor_copy(wkt_rr, wkt_s)
            nc.vector.memset(ebias_s, EXP_SHIFT)
            for a in range(NCHUNK):
                nc.vector.tensor_scalar_mul(
                    gbv_s[:, a, :], bv_s[:, a, :], gb_s
                )

            # ---- big persistent inputs ----
            # xh kept rounded-to-f32r: used as Q-proj rhs and (bitcast
            # f32) as the residual; the tf32-level rounding of X adds
            # ~5e-4 relative noise, well inside tolerance.
            xh_s = bp.tile([128, NCHUNK, NQ], F32R)
            qf_s = bp.tile([CI, NQ], F32R)
            kf_s = bp.tile([CI, N], F32R)
            vt_s = bp.tile([128, NMT, C], BF16)

            with tc.tile_pool(name="ybuf", bufs=1) as yp:
                yb_rr = yp.tile([128, NCHUNK, N], F32R)
                ybh_s = yp.tile([128, NCHUNK, N], BF16)
                with tc.tile_pool(name="ystage", bufs=3) as sp:
                    # X first (smaller): stage -> round, Q-proj starts
                    for g in range(NQ // NG):
                        for a in range(NCHUNK):
                            st = sp.tile([128, NG], F32, tag="st")
                            nc.sync.dma_start(
                                out=st, in_=xh_r[:, a, ts(g, NG)]
                            )
                            nc.vector.tensor_copy(
                                xh_s[:, a, ts(g, NG)], st
                            )

                    # ---- Q projection (f32r) ----
                    for nb in range(NQB):
                        ps = psB.tile([CI, NB], F32, tag="sc")
                        for a in range(NCHUNK):
                            nc.tensor.matmul(
                                out=ps,
                                lhsT=wqt_rr[:, a, :],
                                rhs=xh_s[:, a, ts(nb, NB)],
                                start=(a == 0),
                                stop=(a == NCHUNK - 1),
                            )
                        nc.scalar.activation(
                            qf_s[:, ts(nb, NB)], ps, AF.Identity, bias=bq_s
                        )

                    # Y: stage -> round to f32r + cast to bf16
                    for g in range(N // NG):
                        for a in range(NCHUNK):
                            st = sp.tile([128, NG], F32, tag="st")
                            nc.sync.dma_start(
                                out=st, in_=yb_r[:, a, ts(g, NG)]
                            )
                            nc.vector.tensor_copy(
                                yb_rr[:, a, ts(g, NG)], st
                            )
                            nc.scalar.copy(ybh_s[:, a, ts(g, NG)], st)

                # ---- K projection (f32r) ----
                for nb in range(N // NB):
                    ps = psB.tile([CI, NB], F32, tag="sc")
                    for a in range(NCHUNK):
                        nc.tensor.matmul(
                            out=ps,
                            lhsT=wkt_rr[:, a, :],
                            rhs=yb_rr[:, a, ts(nb, NB)],
                            start=(a == 0),
                            stop=(a == NCHUNK - 1),
                        )
                    nc.scalar.activation(
                        kf_s[:, ts(nb, NB)], ps, AF.Identity, bias=bk_s
                    )

                # fold gamma*bv into the residual input: xh += gamma*bv
                # (after Q-proj consumed the original X)
                for a in range(NCHUNK):
                    nc.vector.tensor_scalar_add(
                        xh_s[:, a, :],
                        xh_s[:, a, :].bitcast(F32),
                        gbv_s[:, a, :],
                    )

                # ---- V^T projection (bf16) ----
                for t in range(NMT):
                    ps = psB.tile([128, NB], F32, tag="sc")
                    for a in range(NCHUNK):
                        nc.tensor.matmul(
                            out=ps,
                            lhsT=ybh_s[:, a, ts(t, MT)],
                            rhs=wvt_s[:, a, :],
                            start=(a == 0),
                            stop=(a == NCHUNK - 1),
                        )
                    if t % 2 == 0:
                        nc.scalar.copy(vt_s[:, t, :], ps)
                    else:
                        nc.vector.tensor_copy(vt_s[:, t, :], ps)

            # ---- attention ----
            with (
                tc.tile_pool(name="es", bufs=6) as ep,
                tc.tile_pool(name="epi", bufs=2) as fp,
            ):
                for q in range(NQB):
                    av = [
                        psA.tile(
                            [128, NB], F32, name=f"av{cc}", tag=f"av{cc}"
                        )
                        for cc in range(NCHUNK)
                    ]
                    acc = fp.tile([128, NB], BF16, tag="acc")

                    def scores(t, q=q):
                        ps = psB.tile([128, NB], F32, tag="sc")
                        nc.tensor.matmul(
                            out=ps,
                            lhsT=kf_s[:, ts(t, MT)],
                            rhs=qf_s[:, ts(q, NB)],
                            start=True,
                            stop=True,
                        )
                        e = ep.tile([128, NB], BF16, tag="es")
                        nc.scalar.activation(e, ps, AF.Exp, bias=ebias_s)
                        return e

                    pipe = [scores(0), scores(1)]
                    tun = []
                    for t in range(NMT):
                        e_cur = pipe.pop(0)
                        if t + 2 < NMT:
                            pipe.append(scores(t + 2))
                        first, last = t == 0, t == NMT - 1
                        for cc in range(NCHUNK):
                            nc.tensor.matmul(
                                out=av[cc],
                                lhsT=vt_s[:, t, ts(cc, 128)],
                                rhs=e_cur,
                                start=first,
                                stop=last,
                            )
                        if last:
                            # release the accumulator banks immediately
                            # (split ACT/DVE, ahead of the colsum tail in
                            # queue order) so the next block's matmuls
                            # don't wait on the normalization chain
                            with tc.high_priority():
                                for cc in range(NCHUNK):
                                    u = fp.tile(
                                        [128, NB], F32,
                                        name=f"tun{cc}", tag=f"tun{cc}",
                                    )
                                    if cc % 2 == 0:
                                        nc.scalar.copy(u, av[cc])
                                    else:
                                        nc.vector.tensor_copy(u, av[cc])
                                    tun.append(u)
                        if first:
                            nc.vector.tensor_copy(acc, e_cur)
                        else:
                            nc.vector.tensor_add(acc, acc, e_cur)

                    # colsum all-reduce (also broadcasts across
                    # partitions) on the otherwise-idle GpSimd engine
                    gsb = fp.tile([128, NB], F32, tag="gsb")
                    nc.gpsimd.partition_all_reduce(
                        gsb, acc, 128, bass_isa.ReduceOp.add
                    )
                    rec = fp.tile([128, NB], F32, tag="rec")
                    nc.vector.reciprocal_approx_fast(rec, gsb)
                    nc.vector.tensor_scalar_mul(rec, rec, gb_s)

                    # epilogue: out = av * (gamma/colsum) + (x+gamma*bv)
                    for cc in range(NCHUNK):
                        t1 = fp.tile([128, NB], F32, tag="t1")
                        nc.vector.tensor_mul(t1, tun[cc], rec)
                        t3 = fp.tile([128, NB], F32, tag="t3")
                        nc.vector.tensor_add(
                            t3, t1, xh_s[:, cc, ts(q, NB)].bitcast(F32)
                        )
                        nc.sync.dma_start(
                            out=out_r[:, cc, ts(q, NB)], in_=t3
                        )
    nc.finalize()
    return nc


_NC = None


def _get_nc():
    global _NC
    if _NC is None:
        _NC = _build()
    return _NC


def _in_maps(X, Y, Wq, bq, Wk, bk, Wv, bv, gamma):
    Xf = np.ascontiguousarray(X, dtype=np.float32).reshape(B, C, N)
    Yf = np.ascontiguousarray(Y, dtype=np.float32).reshape(B, C, N)
    wqt = np.ascontiguousarray(np.asarray(Wq, np.float32).T)
    wkt = np.ascontiguousarray(np.asarray(Wk, np.float32).T)
    wvt = np.ascontiguousarray(np.asarray(Wv, np.float32).T).astype(
        ml_dtypes.bfloat16
    )
    bq2 = np.asarray(bq, np.float32).reshape(CI, 1)
    bk2 = np.asarray(bk, np.float32).reshape(CI, 1)
    bv2 = np.asarray(bv, np.float32).reshape(C, 1)
    g2 = np.asarray(gamma, np.float32).reshape(1, 1)
    maps = []
    for core in range(8):
        b, h = divmod(core, 2)
        maps.append(
            {
                "xh": np.ascontiguousarray(Xf[b, :, h * NQ : (h + 1) * NQ]),
                "yb": np.ascontiguousarray(Yf[b]),
                "wqt": wqt,
                "wkt": wkt,
                "wvt": wvt,
                "bq": bq2,
                "bk": bk2,
                "bv": bv2,
                "gamma": g2,
            }
        )
    return maps


def _run(inputs, trace=False, **kw):
    nc = _get_nc()
    maps = _in_maps(**inputs)
    res = run_bass_kernel_spmd(
        nc, maps, core_ids=list(range(8)), trace=trace, **kw
    )
    out = np.empty((B, C, H, W), np.float32)
    for core in range(8):
        b, h = divmod(core, 2)
        out[b, :, h * (H // 2) : (h + 1) * (H // 2), :] = (
            res.results[core]["out"].reshape(C, H // 2, W)
        )
    return out, res


def kernel(X, Y, Wq, bq, Wk, bk, Wv, bv, gamma):
    out, _ = _run(
        dict(
            X=X, Y=Y, Wq=Wq, bq=bq, Wk=Wk, bk=bk, Wv=Wv, bv=bv, gamma=gamma
        )
    )
    return out


# revision 14
# speedup vs baseline: 1.1599x; 1.1599x over previous
"""Cross-attention Trainium2 kernel (8 NeuronCores, SPMD).

Sharding: core -> (batch b = core//2, query-half h = core%2).
Each core computes out[b, :, h*2048:(h+1)*2048] independently:
  Qf = Wq @ X[b][:, half] + bq          [64, 2048]   (fp16 in, f32 acc)
  Kf = Wk @ Y[b] + bk                   [64, 4096]
  V0t = (Wv @ Y[b])^T                   [4096, 512]  (bf16)
  St[m, n] = sum_c Kf[c, m] Qf[c, n]    (fp16, scores transposed: no
                                         transposes needed anywhere)
  es = exp(St - 40)                     (bf16 for range; the shift
                                         cancels in the normalization)
  AV[c, n] = sum_m V0t[m, c] es[m, n]   (bf16 matmul, fp32 PSUM)
  colsum[n] = sum_m es[m, n]            (DVE accumulate + GpSimd
                                         partition_all_reduce)
  out = AV * (gamma/colsum) + (X + gamma*bv)   (bv folds out of the
                                         attention since sum_m P = 1)
"""

import sys

if "/opt/trn_rl_repo" not in sys.path:
    sys.path.insert(0, "/opt/trn_rl_repo")

import ml_dtypes
import numpy as np

import concourse.bass as bass
import concourse.mybir as mybir
from concourse import bacc, bass_isa
from concourse.bass import ts
from concourse.bass_utils import run_bass_kernel_spmd
from concourse.tile import TileContext

F32 = mybir.dt.float32
F16 = mybir.dt.float16
BF16 = mybir.dt.bfloat16
AF = mybir.ActivationFunctionType

B, C, H, W = 4, 512, 64, 64
CI = 64            # inner q/k dim
N = H * W          # 4096 key positions
NQ = N // 2        # 2048 query positions per core
NB = 512           # n-block (free dim per matmul)
MT = 128           # m-tile (key positions per scores tile)
NCHUNK = C // 128  # 4 channel chunks
NMT = N // MT      # 32 m-tiles
NQB = NQ // NB     # 4 query blocks per core
NG = 512           # dma column group (compute starts early)
EXP_SHIFT = -40.0


def _build():
    nc = bacc.Bacc(
        "TRN2", target_bir_lowering=False, debug=False, num_devices=8
    )
    xh = nc.declare_dram_parameter("xh", [C, NQ], F16, isOutput=False)
    yb = nc.declare_dram_parameter("yb", [C, N], F16, isOutput=False)
    wqt = nc.declare_dram_parameter("wqt", [C, CI], F16, isOutput=False)
    wkt = nc.declare_dram_parameter("wkt", [C, CI], F16, isOutput=False)
    wvt = nc.declare_dram_parameter("wvt", [C, C], F16, isOutput=False)
    bq = nc.declare_dram_parameter("bq", [CI, 1], F32, isOutput=False)
    bk = nc.declare_dram_parameter("bk", [CI, 1], F32, isOutput=False)
    bv = nc.declare_dram_parameter("bv", [C, 1], F32, isOutput=False)
    gamma = nc.declare_dram_parameter("gamma", [1, 1], F32, isOutput=False)
    out = nc.declare_dram_parameter("out", [C, NQ], F32, isOutput=True)

    xh_r = xh.rearrange("(a p) n -> p a n", p=128)
    yb_r = yb.rearrange("(a p) n -> p a n", p=128)
    wqt_r = wqt.rearrange("(a p) m -> p a m", p=128)
    wkt_r = wkt.rearrange("(a p) m -> p a m", p=128)
    wvt_r = wvt.rearrange("(a p) m -> p a m", p=128)
    bv_r = bv.rearrange("(a p) o -> p a o", p=128)
    out_r = out.rearrange("(a p) n -> p a n", p=128)

    with TileContext(nc) as tc:
        with (
            tc.tile_pool(name="weights", bufs=1) as wp,
            tc.tile_pool(name="bigin", bufs=1) as bp,
            tc.tile_pool(name="psA", bufs=1, space="PSUM") as psA,
            tc.tile_pool(name="psB", bufs=4, space="PSUM") as psB,
        ):
            # ---- constants / weights ----
            wqt_s = wp.tile([128, NCHUNK, CI], F16)
            wkt_s = wp.tile([128, NCHUNK, CI], F16)
            wvt_s = wp.tile([128, NCHUNK, C], F16)
            bq_s = wp.tile([CI, 1], F32)
            bk_s = wp.tile([CI, 1], F32)
            bv_s = wp.tile([128, NCHUNK, 1], F32)
            gb_s = wp.tile([128, 1], F32)
            gbv_s = wp.tile([128, NCHUNK, 1], F32)
            ebias_s = wp.tile([128, 1], F32)

            nc.sync.dma_start(out=wqt_s, in_=wqt_r)
            nc.sync.dma_start(out=wkt_s, in_=wkt_r)
            nc.sync.dma_start(out=wvt_s, in_=wvt_r)
            nc.sync.dma_start(out=bq_s, in_=bq[:, :])
            nc.sync.dma_start(out=bk_s, in_=bk[:, :])
            nc.sync.dma_start(out=bv_s, in_=bv_r)
            g_ap = gamma[:, :]
            gb_bcast = bass.AP(
                tensor=g_ap.tensor,
                offset=g_ap.offset,
                ap=[[0, 128]] + list(g_ap.ap)[1:],
            )
            nc.sync.dma_start(out=gb_s, in_=gb_bcast)
            nc.vector.memset(ebias_s, EXP_SHIFT)
            for a in range(NCHUNK):
                nc.vector.tensor_scalar_mul(
                    gbv_s[:, a, :], bv_s[:, a, :], gb_s
                )

            # ---- big persistent inputs (all fp16) ----
            xh_s = bp.tile([128, NCHUNK, NQ], F16)
            yb_s = bp.tile([128, NCHUNK, N], F16)
            qf_s = bp.tile([CI, NQ], F16)
            kf_s = bp.tile([CI, N], F16)
            vt_s = bp.tile([128, NMT, C], BF16)

            for g in range(NQ // NG):
                for a in range(NCHUNK):
                    nc.sync.dma_start(
                        out=xh_s[:, a, ts(g, NG)], in_=xh_r[:, a, ts(g, NG)]
                    )

            # ---- Q projection ----
            for nb in range(NQB):
                ps = psB.tile([CI, NB], F32, tag="sc")
                for a in range(NCHUNK):
                    nc.tensor.matmul(
                        out=ps,
                        lhsT=wqt_s[:, a, :],
                        rhs=xh_s[:, a, ts(nb, NB)],
                        start=(a == 0),
                        stop=(a == NCHUNK - 1),
                    )
                nc.scalar.activation(
                    qf_s[:, ts(nb, NB)], ps, AF.Identity, bias=bq_s
                )

            for g in range(N // NG):
                for a in range(NCHUNK):
                    nc.sync.dma_start(
                        out=yb_s[:, a, ts(g, NG)], in_=yb_r[:, a, ts(g, NG)]
                    )

            # ---- K projection ----
            for nb in range(N // NB):
                ps = psB.tile([CI, NB], F32, tag="sc")
                for a in range(NCHUNK):
                    nc.tensor.matmul(
                        out=ps,
                        lhsT=wkt_s[:, a, :],
                        rhs=yb_s[:, a, ts(nb, NB)],
                        start=(a == 0),
                        stop=(a == NCHUNK - 1),
                    )
                nc.scalar.activation(
                    kf_s[:, ts(nb, NB)], ps, AF.Identity, bias=bk_s
                )

            # fold gamma*bv into the residual input: xh += gamma*bv
            # (after Q-proj consumed the original X)
            for a in range(NCHUNK):
                nc.vector.tensor_scalar_add(
                    xh_s[:, a, :], xh_s[:, a, :], gbv_s[:, a, :]
                )

            # ---- V^T projection ----
            for t in range(NMT):
                ps = psB.tile([128, NB], F32, tag="sc")
                for a in range(NCHUNK):
                    nc.tensor.matmul(
                        out=ps,
                        lhsT=yb_s[:, a, ts(t, MT)],
                        rhs=wvt_s[:, a, :],
                        start=(a == 0),
                        stop=(a == NCHUNK - 1),
                    )
                if t % 2 == 0:
                    nc.scalar.copy(vt_s[:, t, :], ps)
                else:
                    nc.vector.tensor_copy(vt_s[:, t, :], ps)

            # ---- attention ----
            with (
                tc.tile_pool(name="es", bufs=6) as ep,
                tc.tile_pool(name="epi", bufs=2) as fp,
            ):
                for q in range(NQB):
                    av = [
                        psA.tile(
                            [128, NB], F32, name=f"av{cc}", tag=f"av{cc}"
                        )
                        for cc in range(NCHUNK)
                    ]
                    acc = fp.tile([128, NB], BF16, tag="acc")

                    def scores(t, q=q):
                        ps = psB.tile([128, NB], F32, tag="sc")
                        nc.tensor.matmul(
                            out=ps,
                            lhsT=kf_s[:, ts(t, MT)],
                            rhs=qf_s[:, ts(q, NB)],
                            start=True,
                            stop=True,
                        )
                        e = ep.tile([128, NB], BF16, tag="es")
                        nc.scalar.activation(e, ps, AF.Exp, bias=ebias_s)
                        return e

                    pipe = [scores(0), scores(1)]
                    tun = []
                    for t in range(NMT):
                        e_cur = pipe.pop(0)
                        if t + 2 < NMT:
                            pipe.append(scores(t + 2))
                        first, last = t == 0, t == NMT - 1
                        for cc in range(NCHUNK):
                            nc.tensor.matmul(
                                out=av[cc],
                                lhsT=vt_s[:, t, ts(cc, 128)],
                                rhs=e_cur,
                                start=first,
                                stop=last,
                            )
                        if last:
                            # release the accumulator banks immediately
                            # (high priority, split ACT/DVE) so the next
                            # block's matmuls don't wait on the
                            # normalization chain
                            with tc.high_priority():
                                for cc in range(NCHUNK):
                                    u = fp.tile(
                                        [128, NB], F32,
                                        name=f"tun{cc}", tag=f"tun{cc}",
                                    )
                                    if cc % 2 == 0:
                                        nc.scalar.copy(u, av[cc])
                                    else:
                                        nc.vector.tensor_copy(u, av[cc])
                                    tun.append(u)
                        if first:
                            nc.vector.tensor_copy(acc, e_cur)
                        else:
                            nc.vector.tensor_add(acc, acc, e_cur)

                    # colsum all-reduce (also broadcasts across
                    # partitions) on the otherwise-idle GpSimd engine
                    gsb = fp.tile([128, NB], F32, tag="gsb")
                    nc.gpsimd.partition_all_reduce(
                        gsb, acc, 128, bass_isa.ReduceOp.add
                    )
                    rec = fp.tile([128, NB], F32, tag="rec")
                    nc.vector.reciprocal_approx_fast(rec, gsb)
                    nc.vector.tensor_scalar_mul(rec, rec, gb_s)

                    # epilogue: out = av * (gamma/colsum) + (x+gamma*bv)
                    for cc in range(NCHUNK):
                        t1 = fp.tile([128, NB], F32, tag="t1")
                        nc.vector.tensor_mul(t1, tun[cc], rec)
                        t3 = fp.tile([128, NB], F32, tag="t3")
                        nc.vector.tensor_add(
                            t3, t1, xh_s[:, cc, ts(q, NB)]
                        )
                        nc.sync.dma_start(
                            out=out_r[:, cc, ts(q, NB)], in_=t3
                        )
    nc.finalize()
    return nc


_NC = None


def _get_nc():
    global _NC
    if _NC is None:
        _NC = _build()
    return _NC


def _in_maps(X, Y, Wq, bq, Wk, bk, Wv, bv, gamma):
    Xf = np.ascontiguousarray(X, dtype=np.float32).reshape(B, C, N)
    Yf = np.ascontiguousarray(Y, dtype=np.float32).reshape(B, C, N)
    wqt = np.ascontiguousarray(np.asarray(Wq, np.float32).T).astype(
        np.float16
    )
    wkt = np.ascontiguousarray(np.asarray(Wk, np.float32).T).astype(
        np.float16
    )
    wvt = np.ascontiguousarray(np.asarray(Wv, np.float32).T).astype(
        np.float16
    )
    bq2 = np.asarray(bq, np.float32).reshape(CI, 1)
    bk2 = np.asarray(bk, np.float32).reshape(CI, 1)
    bv2 = np.asarray(bv, np.float32).reshape(C, 1)
    g2 = np.asarray(gamma, np.float32).reshape(1, 1)
    maps = []
    for core in range(8):
        b, h = divmod(core, 2)
        maps.append(
            {
                "xh": Xf[b, :, h * NQ : (h + 1) * NQ].astype(np.float16),
                "yb": Yf[b].astype(np.float16),
                "wqt": wqt,
                "wkt": wkt,
                "wvt": wvt,
                "bq": bq2,
                "bk": bk2,
                "bv": bv2,
                "gamma": g2,
            }
        )
    return maps


def _run(inputs, trace=False, **kw):
    nc = _get_nc()
    maps = _in_maps(**inputs)
    res = run_bass_kernel_spmd(
        nc, maps, core_ids=list(range(8)), trace=trace, **kw
    )
    out = np.empty((B, C, H, W), np.float32)
    for core in range(8):
        b, h = divmod(core, 2)
        out[b, :, h * (H // 2) : (h + 1) * (H // 2), :] = (
            res.results[core]["out"].reshape(C, H // 2, W)
        )
    return out, res


def kernel(X, Y, Wq, bq, Wk, bk, Wv, bv, gamma):
    out, _ = _run(
        dict(
            X=X, Y=Y, Wq=Wq, bq=bq, Wk=Wk, bk=bk, Wv=Wv, bv=bv, gamma=gamma
        )
    )
    return out
